# revision 8
# baseline (speedup 1.0000x reference)
"""Causal multi-head attention block (qkv -> attention -> proj) on 8 TRN2 cores.

Problem: x[2,2048,1024], w_qkv[3072,1024], b_qkv[3072], w_proj[1024,1024],
b_proj[1024]; H=16 heads, D=64; softmax scale 1/sqrt(1024).

Sharding: core = (batch b, head-group hg); 2 batches x 4 groups of 4 heads.
Each core computes qkv for its 4 heads, causal attention, and a partial
projection (its heads' columns of w_proj); host sums the 4 partials per batch
and adds b_proj.

On-chip layout: everything the PE contracts over lives partition-major:
x is fed as xT[c,t]; w_qkv/w_proj slices are fed pre-transposed. Attention
computes S^T[s,t] = k^T.T @ q^T directly so softmax stays in the free dim of
nothing -- instead exp is applied unnormalized (scores are O(1) for this
problem) and the softmax denominator comes from a ones-column appended to V.
Division by the denominator is done via a K=1 broadcast matmul + DVE multiply
before the projection. Causality: above-diagonal (s_tile > t_chunk) tiles are
skipped entirely; diagonal tiles are masked with precomputed 0/1 masks.

All matmuls run in float32r (FP22 multiply, FP32 accumulate) at full PE rate.
"""

import os
import sys
import types
import numpy as np
from contextlib import ExitStack

import concourse.bass as bass
import concourse.bacc as bacc
import concourse.tile as tile
import concourse.mybir as mybir
from concourse.bass_utils import run_bass_kernel_spmd

B, T, C, H = 2, 2048, 1024, 16
D = C // H                  # 64, head dim
HPC = 4                     # heads per core
N_CORES = 8
NT = T // 128               # 16 t-tiles of 128
NCT = C // 128              # 8 contraction tiles over C
TCH = T // 512              # 4 t-chunks of 512
SCALE = 1.0 / np.sqrt(np.float32(C))   # 1/32

F32 = mybir.dt.float32
F32R = mybir.dt.float32r
EXP = mybir.ActivationFunctionType.Exp

_CACHE = {}


def _build():
    """Build + compile the SPMD program (identical on all 8 cores)."""
    nc = bacc.Bacc("TRN2", target_bir_lowering=False, debug=False)

    xT = nc.dram_tensor("xT", [C, T], F32R, kind="ExternalInput")          # x[b].T
    wqkT = nc.dram_tensor("wqkT", [C, 2 * HPC * D], F32R, kind="ExternalInput")
    wvT = nc.dram_tensor("wvT", [C, HPC * D], F32R, kind="ExternalInput")
    wpT = nc.dram_tensor("wpT", [HPC * D, C], F32R, kind="ExternalInput")
    bqk = nc.dram_tensor("bqk", [128, 4], F32, kind="ExternalInput")       # per m-tile
    bv = nc.dram_tensor("bv", [1, HPC * D], F32R, kind="ExternalInput")
    ones = nc.dram_tensor("ones", [1, 128], F32R, kind="ExternalInput")
    vones = nc.dram_tensor("vones", [128, 64], F32R, kind="ExternalInput")
    mask = nc.dram_tensor("mask", [128, 2048], F32R, kind="ExternalInput")  # 4x[128,512]
    y = nc.dram_tensor("y", [T, C], F32, kind="ExternalOutput")

    with tile.TileContext(nc) as tc, ExitStack() as ctx:
        sb = ctx.enter_context(tc.tile_pool(name="persist", bufs=1))

        # ---- persistent SBUF tensors ----
        xT_sb = sb.tile([128, NCT * T], F32R, tag="xT")            # [c-tile][t]
        wqk_sb = sb.tile([128, NCT * 512], F32R, tag="wqk")        # [c-tile][m 512]
        wv_sb = sb.tile([128, NCT * 256], F32R, tag="wv")          # [c-tile][m 256]
        wp_sb = sb.tile([128, 2 * C], F32R, tag="wp")              # [ci-tile][co 1024]
        bqk_sb = sb.tile([128, 4], F32, tag="bqk")
        bv_sb = sb.tile([1, HPC * D], F32R, tag="bv")
        ones_sb = sb.tile([1, 128], F32R, tag="ones")
        mask_sb = sb.tile([128, 2048], F32R, tag="mask")
        qk_sb = sb.tile([128, 4 * T], F32R, tag="qk")              # q^T|k^T [m-tile][t]
        v_sb = sb.tile([128, NT * (HPC * (D + 1))], F32R, tag="v")  # per s-tile, +ones col
        on_sb = sb.tile([128, 2 * T], F32R, tag="onorm")           # O_norm^T [ci-tile][t]

        for ct in range(NCT):
            nc.sync.dma_start(xT_sb[:, ct * T:(ct + 1) * T], xT.ap()[ct * 128:(ct + 1) * 128, :])
            nc.sync.dma_start(wqk_sb[:, ct * 512:(ct + 1) * 512], wqkT.ap()[ct * 128:(ct + 1) * 128, :])
            nc.sync.dma_start(wv_sb[:, ct * 256:(ct + 1) * 256], wvT.ap()[ct * 128:(ct + 1) * 128, :])
        for kt in range(2):
            nc.sync.dma_start(wp_sb[:, kt * C:(kt + 1) * C], wpT.ap()[kt * 128:(kt + 1) * 128, :])
        nc.sync.dma_start(bqk_sb[:], bqk.ap())
        nc.sync.dma_start(bv_sb[:], bv.ap())
        nc.sync.dma_start(ones_sb[:], ones.ap())
        nc.sync.dma_start(mask_sb[:], mask.ap())

        VW = HPC * (D + 1)  # 260, per s-tile stride in v_sb

        # v_sb ones columns (softmax denominator trick): col 64 of each head block
        vdst = v_sb[:].rearrange("p (s h e) -> p s h e", s=NT, h=HPC)[:, :, :, D:D + 1]
        vsrc = vones.ap().rearrange("p (s h e) -> p s h e", s=NT, h=HPC)
        nc.sync.dma_start(vdst, vsrc)

        # ---- phase 1: qkv projections ----
        with tc.tile_pool(name="ps1", bufs=4, space="PSUM") as ps1:
            # q^T / k^T: [m, t] layout, m-tiles 0,1 = q (heads 0-3), 2,3 = k
            for mt in range(4):
                for tch in range(TCH):
                    acc = ps1.tile([128, 512], F32, tag="qkacc")
                    for ct in range(NCT):
                        nc.tensor.matmul(
                            acc[:],
                            wqk_sb[:, ct * 512 + mt * 128: ct * 512 + (mt + 1) * 128],
                            xT_sb[:, ct * T + tch * 512: ct * T + tch * 512 + 512],
                            start=(ct == 0), stop=(ct == NCT - 1),
                        )
                    nc.vector.tensor_scalar_add(
                        qk_sb[:, mt * T + tch * 512: mt * T + tch * 512 + 512],
                        acc[:], bqk_sb[:, mt:mt + 1],
                    )
            # v: [t, m] layout (natural for PV stationary operand)
            for st in range(NT):
                acc = ps1.tile([128, 256], F32, tag="vacc")
                nc.tensor.matmul(acc[:], ones_sb[:1, :], bv_sb[:1, :],
                                 start=True, stop=False)
                for ct in range(NCT):
                    nc.tensor.matmul(
                        acc[:],
                        xT_sb[:, ct * T + st * 128: ct * T + (st * 128 + 128)],
                        wv_sb[:, ct * 256:(ct + 1) * 256],
                        start=False, stop=(ct == NCT - 1),
                    )
                dst = v_sb[:, st * VW: st * VW + VW].rearrange(
                    "p (h e) -> p h e", h=HPC)[:, :, 0:D]
                src = acc[:].rearrange("p (h d) -> p h d", h=HPC)
                nc.vector.tensor_copy(dst, src)

        # ---- phase 2: attention (heads packed in pairs via PE row tiling) ----
        with tc.tile_pool(name="ps2", bufs=2, space="PSUM") as ps2, \
             tc.tile_pool(name="ps2b", bufs=2, space="PSUM") as ps2b, \
             tc.tile_pool(name="psacc", bufs=2, space="PSUM") as psacc, \
             tc.tile_pool(name="att", bufs=4) as att:
            for hp in range(2):          # head pair (heads 2hp, 2hp+1)
                qoff = hp * T            # q m-tile = hp
                koff = (2 + hp) * T      # k m-tile = 2+hp
                for tch in range(TCH):
                    n_s = 4 * (tch + 1)
                    acc0 = psacc.tile([128, 512], F32, tag="acc0")
                    acc1 = psacc.tile([128, 512], F32, tag="acc1")
                    for st in range(n_s):
                        s0 = ps2.tile([128, 512], F32, tag="sT0")
                        s1 = ps2b.tile([128, 512], F32, tag="sT1")
                        nc.tensor.matmul(
                            s0[:],
                            qk_sb[0:64, koff + st * 128: koff + st * 128 + 128],
                            qk_sb[0:64, qoff + tch * 512: qoff + tch * 512 + 512],
                            start=True, stop=True, tile_position=(0, 0),
                        )
                        nc.tensor.matmul(
                            s1[:],
                            qk_sb[64:128, koff + st * 128: koff + st * 128 + 128],
                            qk_sb[64:128, qoff + tch * 512: qoff + tch * 512 + 512],
                            start=True, stop=True, tile_position=(64, 0),
                        )
                        p0 = att.tile([128, 512], F32R, tag="p0")
                        p1 = att.tile([128, 512], F32R, tag="p1")
                        nc.scalar.activation(p0[:], s0[:], EXP, scale=float(SCALE))
                        nc.scalar.activation(p1[:], s1[:], EXP, scale=float(SCALE))
                        r = st - 4 * tch
                        if r >= 0:  # diagonal tile: causal 0/1 mask
                            m = mask_sb[:, r * 512:(r + 1) * 512]
                            nc.vector.tensor_mul(p0[:], p0[:], m)
                            nc.vector.tensor_mul(p1[:], p1[:], m)
                        first, last = (st == 0), (st == n_s - 1)
                        nc.tensor.matmul(
                            acc0[0:D + 1, :],
                            v_sb[:, st * VW + (2 * hp) * (D + 1): st * VW + (2 * hp) * (D + 1) + D + 1],
                            p0[:], start=first, stop=last,
                        )
                        nc.tensor.matmul(
                            acc1[0:D + 1, :],
                            v_sb[:, st * VW + (2 * hp + 1) * (D + 1): st * VW + (2 * hp + 1) * (D + 1) + D + 1],
                            p1[:], start=first, stop=last,
                        )
                    # normalize: o / l, write O_norm^T into proj layout
                    for i, acc in ((0, acc0), (1, acc1)):
                        a = 2 * hp + i   # head index in core
                        rl = att.tile([1, 512], F32R, tag="recip")
                        with nc.allow_low_precision(reason="softmax denom recip to f32r"):
                            nc.vector.reciprocal(rl[:], acc[D:D + 1, :])
                        bc = ps2.tile([128, 512], F32, tag="sT0")
                        nc.tensor.matmul(bc[0:D, :], ones_sb[:1, 0:D], rl[:1, :],
                                         start=True, stop=True)
                        ot = att.tile([64, 512], F32, tag="otmp")
                        nc.vector.tensor_copy(ot[:], acc[0:D, :])
                        po = (a % 2) * 64
                        dst = on_sb[po:po + 64,
                                    (a // 2) * T + tch * 512:(a // 2) * T + tch * 512 + 512]
                        nc.vector.tensor_mul(dst, ot[:], bc[0:D, :])

        # ---- phase 3: projection (partial over this core's head columns) ----
        with tc.tile_pool(name="ps3", bufs=4, space="PSUM") as ps3, \
             tc.tile_pool(name="yst", bufs=4) as yst:
            for tt in range(NT):
                for cc in range(2):
                    acc = ps3.tile([128, 512], F32, tag="yacc")
                    for kt in range(2):
                        nc.tensor.matmul(
                            acc[:],
                            on_sb[:, kt * T + tt * 128: kt * T + tt * 128 + 128],
                            wp_sb[:, kt * C + cc * 512: kt * C + cc * 512 + 512],
                            start=(kt == 0), stop=(kt == 1),
                        )
                    ytile = yst.tile([128, 512], F32, tag="ytile")
                    nc.vector.tensor_copy(ytile[:], acc[:])
                    nc.sync.dma_start(
                        y.ap()[tt * 128:(tt + 1) * 128, cc * 512:(cc + 1) * 512],
                        ytile[:],
                    )

    nc.compile()
    return nc


def _causal_masks():
    """mask[p, r*512 + j] = 1.0 if (128*r + p) <= j else 0.0, r in 0..3."""
    p = np.arange(128)[:, None]
    j = np.arange(512)[None, :]
    cols = [((128 * r + p) <= j).astype(np.float32) for r in range(4)]
    return np.concatenate(cols, axis=1)


def _in_maps(x, w_qkv, b_qkv, w_proj):
    mask = _causal_masks()
    ones = np.ones((1, 128), dtype=np.float32)
    maps = []
    for core in range(N_CORES):
        b, hg = divmod(core, 4)
        h0 = hg * HPC                       # first global head of this core
        r0 = h0 * D                         # first q row
        q_w = w_qkv[r0:r0 + HPC * D]                    # [256, C]
        k_w = w_qkv[C + r0:C + r0 + HPC * D]
        v_w = w_qkv[2 * C + r0:2 * C + r0 + HPC * D]
        wqkT = np.ascontiguousarray(np.concatenate([q_w, k_w], axis=0).T)  # [C, 512]
        wvT = np.ascontiguousarray(v_w.T)                                  # [C, 256]
        wpT = np.ascontiguousarray(w_proj[:, r0:r0 + HPC * D].T)           # [256, C]
        bq = b_qkv[r0:r0 + HPC * D]
        bk = b_qkv[C + r0:C + r0 + HPC * D]
        bqk = np.ascontiguousarray(
            np.concatenate([bq, bk]).reshape(4, 128).T)                    # [128, 4]
        bv = np.ascontiguousarray(b_qkv[2 * C + r0:2 * C + r0 + HPC * D].reshape(1, -1))
        maps.append({
            "xT": np.ascontiguousarray(x[b].T),
            "wqkT": wqkT,
            "wvT": wvT,
            "wpT": wpT,
            "bqk": bqk,
            "bv": bv,
            "ones": ones,
            "vones": np.ones((128, 64), dtype=np.float32),
            "mask": mask,
        })
    return maps


def kernel(x, w_qkv, b_qkv, w_proj, b_proj, _trace=False, _tmpdir=None):
    x = np.asarray(x, dtype=np.float32)
    w_qkv = np.asarray(w_qkv, dtype=np.float32)
    b_qkv = np.asarray(b_qkv, dtype=np.float32)
    w_proj = np.asarray(w_proj, dtype=np.float32)
    b_proj = np.asarray(b_proj, dtype=np.float32)

    if "nc" not in _CACHE:
        _CACHE["nc"] = _build()
    nc = _CACHE["nc"]

    maps = _in_maps(x, w_qkv, b_qkv, w_proj)
    kw = {}
    if _trace:
        kw = {"trace": True, "tmpdir": _tmpdir}
    res = run_bass_kernel_spmd(nc, maps, list(range(N_CORES)), **kw)

    out = np.empty((B, T, C), dtype=np.float32)
    for b in range(B):
        acc = res.results[4 * b]["y"].astype(np.float32)
        for hg in range(1, 4):
            acc = acc + res.results[4 * b + hg]["y"]
        out[b] = acc + b_proj[None, :]
    if _trace:
        return out, res
    return out


# revision 11
# speedup vs baseline: 1.3117x; 1.3117x over previous
"""Causal multi-head attention block (qkv -> attention -> proj) on 8 TRN2 cores.

Problem: x[2,2048,1024], w_qkv[3072,1024], b_qkv[3072], w_proj[1024,1024],
b_proj[1024]; H=16 heads, D=64; softmax scale 1/sqrt(1024).

Sharding: core = (batch b, head-group hg); 2 batches x 4 groups of 4 heads.
Each core computes qkv for its 4 heads, causal attention, and a partial
projection (its heads' columns of w_proj); host sums the 4 partials per batch
and adds b_proj.

Everything the PE contracts over lives partition-major: x is fed as xT[c,t];
weights are fed pre-transposed. Attention computes S^T[s,t] = k^T.T @ q^T
directly (no transposes anywhere), exp is applied unnormalized (scores are
O(1) here), and V is augmented with 64 ones-columns so the PV matmul yields
the softmax denominator replicated across partitions 64..127 -- normalization
is then one reciprocal_approx_fast + one DVE multiply per (head, chunk).
Causality: above-diagonal s-tiles are skipped; diagonal slabs are masked with
precomputed 0/1 masks. QK^T packs two heads in the PE via row tiling (K=64).

All matmuls run in float32r (FP22 multiply, FP32 accumulate) at full PE rate.
"""

import numpy as np
from contextlib import ExitStack

import concourse.bass as bass
import concourse.bacc as bacc
import concourse.tile as tile
import concourse.mybir as mybir
from concourse.bass_utils import run_bass_kernel_spmd

B, T, C, H = 2, 2048, 1024, 16
D = C // H                  # 64, head dim
HPC = 4                     # heads per core
N_CORES = 8
NT = T // 128               # 16 t-tiles / s-tiles of 128
NCT = C // 128              # 8 contraction tiles over C
TCH = T // 512              # 4 t-chunks of 512
SCALE = 1.0 / np.sqrt(np.float32(C))   # 1/32

F32 = mybir.dt.float32
F32R = mybir.dt.float32r
EXP = mybir.ActivationFunctionType.Exp
COPY = mybir.ActivationFunctionType.Copy

VW = 2 * D                  # 128: per-head block in v_sb = [v_h (64) | ones (64)]

_CACHE = {}


def _build():
    """Build + compile the SPMD program (identical on all 8 cores)."""
    nc = bacc.Bacc("TRN2", target_bir_lowering=False, debug=False)

    xT = nc.dram_tensor("xT", [C, T], F32R, kind="ExternalInput")          # x[b].T
    wqkT = nc.dram_tensor("wqkT", [C, 2 * HPC * D], F32R, kind="ExternalInput")
    wvT = nc.dram_tensor("wvT", [C, HPC * D], F32R, kind="ExternalInput")
    wpT = nc.dram_tensor("wpT", [HPC * D, C], F32R, kind="ExternalInput")
    bqk = nc.dram_tensor("bqk", [128, 4], F32, kind="ExternalInput")       # per m-tile
    bv = nc.dram_tensor("bv", [1, HPC * D], F32R, kind="ExternalInput")
    ones = nc.dram_tensor("ones", [1, 128], F32R, kind="ExternalInput")
    vones = nc.dram_tensor("vones", [128, NT * HPC * D], F32R, kind="ExternalInput")
    mask = nc.dram_tensor("mask", [128, 2048], F32R, kind="ExternalInput")  # 4x[128,512]
    y = nc.dram_tensor("y", [T, C], F32, kind="ExternalOutput")

    with tile.TileContext(nc) as tc, ExitStack() as ctx:
        sb = ctx.enter_context(tc.tile_pool(name="persist", bufs=1))

        # ---- persistent SBUF tensors ----
        wqk_sb = sb.tile([128, NCT * 512], F32R, tag="wqk")        # [c-tile][m 512]
        wv_sb = sb.tile([128, NCT * 256], F32R, tag="wv")          # [c-tile][m 256]
        wp_sb = sb.tile([128, 2 * C], F32R, tag="wp")              # [ci-tile][co 1024]
        bqk_sb = sb.tile([128, 4], F32, tag="bqk")
        bv_sb = sb.tile([1, HPC * D], F32R, tag="bv")
        ones_sb = sb.tile([1, 128], F32R, tag="ones")
        mask_sb = sb.tile([128, 2048], F32R, tag="mask")
        qk_sb = sb.tile([128, 4 * T], F32R, tag="qk")              # q^T|k^T [m-tile][t]
        v_sb = sb.tile([128, NT * HPC * VW], F32R, tag="v")        # [s-tile][h][v|ones]
        on_sb = sb.tile([128, 2 * T], F32R, tag="onorm")           # O_norm^T [ci-tile][t]

        for ct in range(NCT):
            nc.sync.dma_start(wqk_sb[:, ct * 512:(ct + 1) * 512], wqkT.ap()[ct * 128:(ct + 1) * 128, :])
            nc.sync.dma_start(wv_sb[:, ct * 256:(ct + 1) * 256], wvT.ap()[ct * 128:(ct + 1) * 128, :])
        for kt in range(2):
            nc.sync.dma_start(wp_sb[:, kt * C:(kt + 1) * C], wpT.ap()[kt * 128:(kt + 1) * 128, :])
        nc.sync.dma_start(bqk_sb[:], bqk.ap())
        nc.sync.dma_start(bv_sb[:], bv.ap())
        nc.sync.dma_start(ones_sb[:], ones.ap())
        nc.sync.dma_start(mask_sb[:], mask.ap())
        # ones columns of v_sb (softmax denominator trick), cols 64..127 per head
        vdst = v_sb[:].rearrange("p (s h e) -> p s h e", s=NT, h=HPC)[:, :, :, D:VW]
        vsrc = vones.ap().rearrange("p (s h e) -> p s h e", s=NT, h=HPC)
        nc.sync.dma_start(vdst, vsrc)

        # ---- phase 1: qkv projections ----
        with tc.tile_pool(name="xTp", bufs=1) as xtp, \
             tc.tile_pool(name="ps1", bufs=4, space="PSUM") as ps1:
            xT_sb = xtp.tile([128, NCT * T], F32R, tag="xT")       # [c-tile][t]
            for ct in range(NCT):
                nc.sync.dma_start(xT_sb[:, ct * T:(ct + 1) * T], xT.ap()[ct * 128:(ct + 1) * 128, :])

            # q^T / k^T: [m, t] layout, m-tiles 0,1 = q (heads 0-3), 2,3 = k
            for mt in range(4):
                for tch in range(TCH):
                    acc = ps1.tile([128, 512], F32, tag="qkacc")
                    for ct in range(NCT):
                        nc.tensor.matmul(
                            acc[:],
                            wqk_sb[:, ct * 512 + mt * 128: ct * 512 + (mt + 1) * 128],
                            xT_sb[:, ct * T + tch * 512: ct * T + tch * 512 + 512],
                            start=(ct == 0), stop=(ct == NCT - 1),
                        )
                    nc.vector.tensor_scalar_add(
                        qk_sb[:, mt * T + tch * 512: mt * T + tch * 512 + 512],
                        acc[:], bqk_sb[:, mt:mt + 1],
                    )
            # v: [t, m] layout (natural for PV stationary operand)
            for st in range(NT):
                acc = ps1.tile([128, 256], F32, tag="vacc")
                nc.tensor.matmul(acc[:], ones_sb[:1, :], bv_sb[:1, :],
                                 start=True, stop=False)
                for ct in range(NCT):
                    nc.tensor.matmul(
                        acc[:],
                        xT_sb[:, ct * T + st * 128: ct * T + (st * 128 + 128)],
                        wv_sb[:, ct * 256:(ct + 1) * 256],
                        start=False, stop=(ct == NCT - 1),
                    )
                dst = v_sb[:, st * HPC * VW: (st + 1) * HPC * VW].rearrange(
                    "p (h e) -> p h e", h=HPC)[:, :, 0:D]
                src = acc[:].rearrange("p (h d) -> p h d", h=HPC)
                nc.vector.tensor_copy(dst, src)

        # ---- phase 2+3: attention (t-chunk outer, proj interleaved) ----
        with tc.tile_pool(name="ps2", bufs=1, space="PSUM") as ps2, \
             tc.tile_pool(name="ps2b", bufs=1, space="PSUM") as ps2b, \
             tc.tile_pool(name="psacc", bufs=1, space="PSUM") as psacc, \
             tc.tile_pool(name="ps3", bufs=2, space="PSUM") as ps3, \
             tc.tile_pool(name="att", bufs=3) as att, \
             tc.tile_pool(name="yst", bufs=4) as yst:
            for tch in range(TCH):
                for hp in range(2):      # head pair (heads 2hp, 2hp+1)
                    qoff = hp * T        # q m-tile = hp
                    koff = (2 + hp) * T  # k m-tile = 2+hp
                    acc0 = psacc.tile([128, 512], F32, tag="acc0")
                    acc1 = psacc.tile([128, 512], F32, tag="acc1")
                    n_slab = 2 * (tch + 1)
                    for g in range(n_slab):
                        sG0 = ps2.tile([128, 1024], F32, tag="sG0")
                        sG1 = ps2b.tile([128, 1024], F32, tag="sG1")
                        for j in range(2):
                            st = 2 * g + j
                            nc.tensor.matmul(
                                sG0[:, j * 512:(j + 1) * 512],
                                qk_sb[0:64, koff + st * 128: koff + st * 128 + 128],
                                qk_sb[0:64, qoff + tch * 512: qoff + tch * 512 + 512],
                                start=True, stop=True, tile_position=(0, 0),
                            )
                            nc.tensor.matmul(
                                sG1[:, j * 512:(j + 1) * 512],
                                qk_sb[64:128, koff + st * 128: koff + st * 128 + 128],
                                qk_sb[64:128, qoff + tch * 512: qoff + tch * 512 + 512],
                                start=True, stop=True, tile_position=(64, 0),
                            )
                        p0 = att.tile([128, 1024], F32R, tag="p0")
                        p1 = att.tile([128, 1024], F32R, tag="p1")
                        nc.scalar.activation(p0[:], sG0[:], EXP, scale=float(SCALE))
                        nc.scalar.activation(p1[:], sG1[:], EXP, scale=float(SCALE))
                        if g >= 2 * tch:   # diagonal slab: causal 0/1 mask
                            mi = (g - 2 * tch) * 1024
                            m = mask_sb[:, mi:mi + 1024]
                            nc.vector.tensor_mul(p0[:], p0[:], m)
                            nc.vector.tensor_mul(p1[:], p1[:], m)
                        first, last = (g == 0), (g == n_slab - 1)
                        for j in range(2):
                            st = 2 * g + j
                            nc.tensor.matmul(
                                acc0[:],
                                v_sb[:, st * HPC * VW + (2 * hp) * VW: st * HPC * VW + (2 * hp) * VW + VW],
                                p0[:, j * 512:(j + 1) * 512],
                                start=(first and j == 0), stop=(last and j == 1),
                            )
                            nc.tensor.matmul(
                                acc1[:],
                                v_sb[:, st * HPC * VW + (2 * hp + 1) * VW: st * HPC * VW + (2 * hp + 1) * VW + VW],
                                p1[:, j * 512:(j + 1) * 512],
                                start=(first and j == 0), stop=(last and j == 1),
                            )
                    # normalize: O_norm^T = O^T * (1/l), l replicated on rows 64..127
                    for i, acc in ((0, acc0), (1, acc1)):
                        a = 2 * hp + i   # head index in core
                        # full-tile recip: custom-DVE op rejects partition
                        # slices; rows 0..63 compute garbage and are unused
                        rl = att.tile([128, 512], F32, tag="rl")
                        nc.vector.reciprocal_approx_fast(rl[:], acc[:])
                        po = (a % 2) * 64
                        dst = on_sb[po:po + 64,
                                    (a // 2) * T + tch * 512:(a // 2) * T + tch * 512 + 512]
                        nc.vector.tensor_mul(dst, acc[0:D, :], rl[64:128, :])

                # proj for this t-chunk (needs all 4 heads' O_norm^T at these t)
                for tt in range(4 * tch, 4 * tch + 4):
                    for cc in range(2):
                        acc = ps3.tile([128, 512], F32, tag="yacc")
                        for kt in range(2):
                            nc.tensor.matmul(
                                acc[:],
                                on_sb[:, kt * T + tt * 128: kt * T + tt * 128 + 128],
                                wp_sb[:, kt * C + cc * 512: kt * C + cc * 512 + 512],
                                start=(kt == 0), stop=(kt == 1),
                            )
                        ytile = yst.tile([128, 512], F32, tag="ytile")
                        if cc == 0:
                            nc.vector.tensor_copy(ytile[:], acc[:])
                        else:
                            nc.scalar.activation(ytile[:], acc[:], COPY)
                        nc.sync.dma_start(
                            y.ap()[tt * 128:(tt + 1) * 128, cc * 512:(cc + 1) * 512],
                            ytile[:],
                        )

    nc.compile()
    return nc


def _causal_masks():
    """mask[p, r*512 + j] = 1.0 if (128*r + p) <= j else 0.0, r in 0..3."""
    p = np.arange(128)[:, None]
    j = np.arange(512)[None, :]
    cols = [((128 * r + p) <= j).astype(np.float32) for r in range(4)]
    return np.concatenate(cols, axis=1)


def _in_maps(x, w_qkv, b_qkv, w_proj):
    mask = _causal_masks()
    ones = np.ones((1, 128), dtype=np.float32)
    vones = np.ones((128, NT * HPC * D), dtype=np.float32)
    maps = []
    for core in range(N_CORES):
        b, hg = divmod(core, 4)
        h0 = hg * HPC                       # first global head of this core
        r0 = h0 * D                         # first q row
        q_w = w_qkv[r0:r0 + HPC * D]                    # [256, C]
        k_w = w_qkv[C + r0:C + r0 + HPC * D]
        v_w = w_qkv[2 * C + r0:2 * C + r0 + HPC * D]
        wqkT = np.ascontiguousarray(np.concatenate([q_w, k_w], axis=0).T)  # [C, 512]
        wvT = np.ascontiguousarray(v_w.T)                                  # [C, 256]
        wpT = np.ascontiguousarray(w_proj[:, r0:r0 + HPC * D].T)           # [256, C]
        bq = b_qkv[r0:r0 + HPC * D]
        bk = b_qkv[C + r0:C + r0 + HPC * D]
        bqk = np.ascontiguousarray(
            np.concatenate([bq, bk]).reshape(4, 128).T)                    # [128, 4]
        bv = np.ascontiguousarray(b_qkv[2 * C + r0:2 * C + r0 + HPC * D].reshape(1, -1))
        maps.append({
            "xT": np.ascontiguousarray(x[b].T),
            "wqkT": wqkT,
            "wvT": wvT,
            "wpT": wpT,
            "bqk": bqk,
            "bv": bv,
            "ones": ones,
            "vones": vones,
            "mask": mask,
        })
    return maps


def kernel(x, w_qkv, b_qkv, w_proj, b_proj, _trace=False, _tmpdir=None):
    x = np.asarray(x, dtype=np.float32)
    w_qkv = np.asarray(w_qkv, dtype=np.float32)
    b_qkv = np.asarray(b_qkv, dtype=np.float32)
    w_proj = np.asarray(w_proj, dtype=np.float32)
    b_proj = np.asarray(b_proj, dtype=np.float32)

    if "nc" not in _CACHE:
        _CACHE["nc"] = _build()
    nc = _CACHE["nc"]

    maps = _in_maps(x, w_qkv, b_qkv, w_proj)
    kw = {}
    if _trace:
        kw = {"trace": True, "tmpdir": _tmpdir}
    res = run_bass_kernel_spmd(nc, maps, list(range(N_CORES)), **kw)

    out = np.empty((B, T, C), dtype=np.float32)
    for b in range(B):
        acc = res.results[4 * b]["y"].astype(np.float32)
        for hg in range(1, 4):
            acc = acc + res.results[4 * b + hg]["y"]
        out[b] = acc + b_proj[None, :]
    if _trace:
        return out, res
    return out


# revision 12
# speedup vs baseline: 1.4777x; 1.1266x over previous
"""Causal multi-head attention block (qkv -> attention -> proj) on 8 TRN2 cores.

Problem: x[2,2048,1024], w_qkv[3072,1024], b_qkv[3072], w_proj[1024,1024],
b_proj[1024]; H=16 heads, D=64; softmax scale 1/sqrt(1024).

Sharding: core = (batch b, head-group hg); 2 batches x 4 groups of 4 heads.
Each core computes qkv for its 4 heads, causal attention, and a partial
projection (its heads' columns of w_proj); host sums the 4 partials per batch
and adds b_proj.

Everything the PE contracts over lives partition-major: x is fed as xT[c,t];
weights are fed pre-transposed. Attention computes S^T[s,t] = k^T.T @ q^T
directly (no transposes anywhere), exp is applied unnormalized (scores are
O(1) here), and V is augmented with 64 ones-columns so the PV matmul yields
the softmax denominator replicated across partitions 64..127 -- normalization
is then one reciprocal_approx_fast + one DVE multiply per (head, chunk).
Causality: above-diagonal s-tiles are skipped; diagonal slabs are masked with
precomputed 0/1 masks. QK^T packs two heads in the PE via row tiling (K=64).

All matmuls run in float32r (FP22 multiply, FP32 accumulate) at full PE rate.
"""

import numpy as np
import ml_dtypes
from contextlib import ExitStack

import concourse.bass as bass
import concourse.bacc as bacc
import concourse.tile as tile
import concourse.mybir as mybir
from concourse.bass_utils import run_bass_kernel_spmd

B, T, C, H = 2, 2048, 1024, 16
D = C // H                  # 64, head dim
HPC = 4                     # heads per core
N_CORES = 8
NT = T // 128               # 16 t-tiles / s-tiles of 128
NCT = C // 128              # 8 contraction tiles over C
TCH = T // 512              # 4 t-chunks of 512
SCALE = 1.0 / np.sqrt(np.float32(C))   # 1/32

F32 = mybir.dt.float32
F32R = mybir.dt.float32r
EXP = mybir.ActivationFunctionType.Exp
BF16 = mybir.dt.bfloat16
COPY = mybir.ActivationFunctionType.Copy

VW = 2 * D                  # 128: per-head block in v_sb = [v_h (64) | ones (64)]

_CACHE = {}


def _build():
    """Build + compile the SPMD program (identical on all 8 cores)."""
    nc = bacc.Bacc("TRN2", target_bir_lowering=False, debug=False)

    xT = nc.dram_tensor("xT", [C, T], F32R, kind="ExternalInput")          # x[b].T
    wqkT = nc.dram_tensor("wqkT", [C, 2 * HPC * D], F32R, kind="ExternalInput")
    wvT = nc.dram_tensor("wvT", [C, HPC * D], F32R, kind="ExternalInput")
    wpT = nc.dram_tensor("wpT", [HPC * D, C], F32R, kind="ExternalInput")
    bqk = nc.dram_tensor("bqk", [128, 4], F32, kind="ExternalInput")       # per m-tile
    bv = nc.dram_tensor("bv", [1, HPC * D], F32R, kind="ExternalInput")
    ones = nc.dram_tensor("ones", [1, 128], F32R, kind="ExternalInput")
    vones = nc.dram_tensor("vones", [128, NT * HPC * D], BF16, kind="ExternalInput")
    mask = nc.dram_tensor("mask", [128, 2048], BF16, kind="ExternalInput")  # 4x[128,512]
    y = nc.dram_tensor("y", [T, C], F32, kind="ExternalOutput")

    with tile.TileContext(nc) as tc, ExitStack() as ctx:
        sb = ctx.enter_context(tc.tile_pool(name="persist", bufs=1))

        # ---- persistent SBUF tensors ----
        wqk_sb = sb.tile([128, NCT * 512], F32R, tag="wqk")        # [c-tile][m 512]
        wv_sb = sb.tile([128, NCT * 256], F32R, tag="wv")          # [c-tile][m 256]
        wp_sb = sb.tile([128, 2 * C], F32R, tag="wp")              # [ci-tile][co 1024]
        bqk_sb = sb.tile([128, 4], F32, tag="bqk")
        bv_sb = sb.tile([1, HPC * D], F32R, tag="bv")
        ones_sb = sb.tile([1, 128], F32R, tag="ones")
        mask_sb = sb.tile([128, 2048], BF16, tag="mask")
        qk_sb = sb.tile([128, 4 * T], BF16, tag="qk")              # q^T|k^T [m-tile][t]
        v_sb = sb.tile([128, NT * HPC * VW], BF16, tag="v")        # [s-tile][h][v|ones]
        on_sb = sb.tile([128, 2 * T], F32R, tag="onorm")           # O_norm^T [ci-tile][t]

        nc.sync.dma_start(bqk_sb[:], bqk.ap())
        nc.sync.dma_start(bv_sb[:], bv.ap())
        nc.sync.dma_start(ones_sb[:], ones.ap())
        for kt in range(2):
            nc.sync.dma_start(wp_sb[:, kt * C:(kt + 1) * C], wpT.ap()[kt * 128:(kt + 1) * 128, :])
        nc.sync.dma_start(mask_sb[:], mask.ap())
        # ones columns of v_sb (softmax denominator trick), cols 64..127 per head
        vdst = v_sb[:].rearrange("p (s h e) -> p s h e", s=NT, h=HPC)[:, :, :, D:VW]
        vsrc = vones.ap().rearrange("p (s h e) -> p s h e", s=NT, h=HPC)
        nc.sync.dma_start(vdst, vsrc)

        # ---- phase 1: qkv projections ----
        with tc.tile_pool(name="xTp", bufs=1) as xtp, \
             tc.tile_pool(name="ps1", bufs=4, space="PSUM") as ps1:
            xT_sb = xtp.tile([128, NCT * T], F32R, tag="xT")       # [c-tile][t]
            for ct in range(NCT):
                nc.sync.dma_start(wqk_sb[:, ct * 512:(ct + 1) * 512], wqkT.ap()[ct * 128:(ct + 1) * 128, :])
                nc.sync.dma_start(xT_sb[:, ct * T:(ct + 1) * T], xT.ap()[ct * 128:(ct + 1) * 128, :])
                nc.sync.dma_start(wv_sb[:, ct * 256:(ct + 1) * 256], wvT.ap()[ct * 128:(ct + 1) * 128, :])

            # q^T / k^T: [m, t] layout, m-tiles 0,1 = q (heads 0-3), 2,3 = k
            for mt in range(4):
                for tch in range(TCH):
                    acc = ps1.tile([128, 512], F32, tag="qkacc")
                    for ct in range(NCT):
                        nc.tensor.matmul(
                            acc[:],
                            wqk_sb[:, ct * 512 + mt * 128: ct * 512 + (mt + 1) * 128],
                            xT_sb[:, ct * T + tch * 512: ct * T + tch * 512 + 512],
                            start=(ct == 0), stop=(ct == NCT - 1),
                        )
                    nc.vector.tensor_scalar_add(
                        qk_sb[:, mt * T + tch * 512: mt * T + tch * 512 + 512],
                        acc[:], bqk_sb[:, mt:mt + 1],
                    )
            # v: [t, m] layout (natural for PV stationary operand)
            for st in range(NT):
                acc = ps1.tile([128, 256], F32, tag="vacc")
                nc.tensor.matmul(acc[:], ones_sb[:1, :], bv_sb[:1, :],
                                 start=True, stop=False)
                for ct in range(NCT):
                    nc.tensor.matmul(
                        acc[:],
                        xT_sb[:, ct * T + st * 128: ct * T + (st * 128 + 128)],
                        wv_sb[:, ct * 256:(ct + 1) * 256],
                        start=False, stop=(ct == NCT - 1),
                    )
                dst = v_sb[:, st * HPC * VW: (st + 1) * HPC * VW].rearrange(
                    "p (h e) -> p h e", h=HPC)[:, :, 0:D]
                src = acc[:].rearrange("p (h d) -> p h d", h=HPC)
                nc.vector.tensor_copy(dst, src)

        # ---- phase 2+3: attention (t-chunk outer, proj interleaved) ----
        with tc.tile_pool(name="ps2", bufs=1, space="PSUM") as ps2, \
             tc.tile_pool(name="ps2b", bufs=1, space="PSUM") as ps2b, \
             tc.tile_pool(name="psacc", bufs=1, space="PSUM") as psacc, \
             tc.tile_pool(name="ps3", bufs=2, space="PSUM") as ps3, \
             tc.tile_pool(name="att", bufs=3) as att, \
             tc.tile_pool(name="yst", bufs=4) as yst:
            for tch in range(TCH):
                for hp in range(2):      # head pair (heads 2hp, 2hp+1)
                    qoff = hp * T        # q m-tile = hp
                    koff = (2 + hp) * T  # k m-tile = 2+hp
                    acc0 = psacc.tile([128, 512], F32, tag="acc0")
                    acc1 = psacc.tile([128, 512], F32, tag="acc1")
                    n_slab = 2 * (tch + 1)
                    for g in range(n_slab):
                        sG0 = ps2.tile([128, 1024], F32, tag="sG0")
                        sG1 = ps2b.tile([128, 1024], F32, tag="sG1")
                        for j in range(2):
                            st = 2 * g + j
                            nc.tensor.matmul(
                                sG0[:, j * 512:(j + 1) * 512],
                                qk_sb[0:64, koff + st * 128: koff + st * 128 + 128],
                                qk_sb[0:64, qoff + tch * 512: qoff + tch * 512 + 512],
                                start=True, stop=True, tile_position=(0, 0),
                            )
                            nc.tensor.matmul(
                                sG1[:, j * 512:(j + 1) * 512],
                                qk_sb[64:128, koff + st * 128: koff + st * 128 + 128],
                                qk_sb[64:128, qoff + tch * 512: qoff + tch * 512 + 512],
                                start=True, stop=True, tile_position=(64, 0),
                            )
                        p0 = att.tile([128, 1024], BF16, tag="p0")
                        p1 = att.tile([128, 1024], BF16, tag="p1")
                        nc.scalar.activation(p0[:], sG0[:], EXP, scale=float(SCALE))
                        nc.scalar.activation(p1[:], sG1[:], EXP, scale=float(SCALE))
                        if g >= 2 * tch:   # diagonal slab: causal 0/1 mask
                            mi = (g - 2 * tch) * 1024
                            m = mask_sb[:, mi:mi + 1024]
                            nc.vector.tensor_mul(p0[:], p0[:], m)
                            nc.vector.tensor_mul(p1[:], p1[:], m)
                        first, last = (g == 0), (g == n_slab - 1)
                        for j in range(2):
                            st = 2 * g + j
                            nc.tensor.matmul(
                                acc0[:],
                                v_sb[:, st * HPC * VW + (2 * hp) * VW: st * HPC * VW + (2 * hp) * VW + VW],
                                p0[:, j * 512:(j + 1) * 512],
                                start=(first and j == 0), stop=(last and j == 1),
                            )
                            nc.tensor.matmul(
                                acc1[:],
                                v_sb[:, st * HPC * VW + (2 * hp + 1) * VW: st * HPC * VW + (2 * hp + 1) * VW + VW],
                                p1[:, j * 512:(j + 1) * 512],
                                start=(first and j == 0), stop=(last and j == 1),
                            )
                    # normalize: O_norm^T = O^T * (1/l), l replicated on rows 64..127
                    for i, acc in ((0, acc0), (1, acc1)):
                        a = 2 * hp + i   # head index in core
                        # full-tile recip: custom-DVE op rejects partition
                        # slices; rows 0..63 compute garbage and are unused
                        rl = att.tile([128, 512], F32, tag="rl")
                        nc.vector.reciprocal_approx_fast(rl[:], acc[:])
                        po = (a % 2) * 64
                        dst = on_sb[po:po + 64,
                                    (a // 2) * T + tch * 512:(a // 2) * T + tch * 512 + 512]
                        nc.vector.tensor_mul(dst, acc[0:D, :], rl[64:128, :])

                # proj for this t-chunk (needs all 4 heads' O_norm^T at these t)
                for tt in range(4 * tch, 4 * tch + 4):
                    for cc in range(2):
                        acc = ps3.tile([128, 512], F32, tag="yacc")
                        for kt in range(2):
                            nc.tensor.matmul(
                                acc[:],
                                on_sb[:, kt * T + tt * 128: kt * T + tt * 128 + 128],
                                wp_sb[:, kt * C + cc * 512: kt * C + cc * 512 + 512],
                                start=(kt == 0), stop=(kt == 1),
                            )
                        ytile = yst.tile([128, 512], F32, tag="ytile")
                        nc.vector.tensor_copy(ytile[:], acc[:])
                        nc.sync.dma_start(
                            y.ap()[tt * 128:(tt + 1) * 128, cc * 512:(cc + 1) * 512],
                            ytile[:],
                        )

    nc.compile()
    return nc


def _causal_masks():
    """mask[p, r*512 + j] = 1.0 if (128*r + p) <= j else 0.0, r in 0..3."""
    p = np.arange(128)[:, None]
    j = np.arange(512)[None, :]
    cols = [((128 * r + p) <= j).astype(np.float32) for r in range(4)]
    return np.concatenate(cols, axis=1)


def _in_maps(x, w_qkv, b_qkv, w_proj):
    mask = _causal_masks()
    ones = np.ones((1, 128), dtype=np.float32)
    vones = np.ones((128, NT * HPC * D), dtype=np.float32)  # cast below
    maps = []
    for core in range(N_CORES):
        b, hg = divmod(core, 4)
        h0 = hg * HPC                       # first global head of this core
        r0 = h0 * D                         # first q row
        q_w = w_qkv[r0:r0 + HPC * D]                    # [256, C]
        k_w = w_qkv[C + r0:C + r0 + HPC * D]
        v_w = w_qkv[2 * C + r0:2 * C + r0 + HPC * D]
        wqkT = np.ascontiguousarray(np.concatenate([q_w, k_w], axis=0).T)  # [C, 512]
        wvT = np.ascontiguousarray(v_w.T)                                  # [C, 256]
        wpT = np.ascontiguousarray(w_proj[:, r0:r0 + HPC * D].T)           # [256, C]
        bq = b_qkv[r0:r0 + HPC * D]
        bk = b_qkv[C + r0:C + r0 + HPC * D]
        bqk = np.ascontiguousarray(
            np.concatenate([bq, bk]).reshape(4, 128).T)                    # [128, 4]
        bv = np.ascontiguousarray(b_qkv[2 * C + r0:2 * C + r0 + HPC * D].reshape(1, -1))
        maps.append({
            "xT": np.ascontiguousarray(x[b].T),
            "wqkT": wqkT,
            "wvT": wvT,
            "wpT": wpT,
            "bqk": bqk,
            "bv": bv,
            "ones": ones,
            "vones": vones.astype(ml_dtypes.bfloat16),
            "mask": mask.astype(ml_dtypes.bfloat16),
        })
    return maps


def kernel(x, w_qkv, b_qkv, w_proj, b_proj, _trace=False, _tmpdir=None):
    x = np.asarray(x, dtype=np.float32)
    w_qkv = np.asarray(w_qkv, dtype=np.float32)
    b_qkv = np.asarray(b_qkv, dtype=np.float32)
    w_proj = np.asarray(w_proj, dtype=np.float32)
    b_proj = np.asarray(b_proj, dtype=np.float32)

    if "nc" not in _CACHE:
        _CACHE["nc"] = _build()
    nc = _CACHE["nc"]

    maps = _in_maps(x, w_qkv, b_qkv, w_proj)
    kw = {}
    if _trace:
        kw = {"trace": True, "tmpdir": _tmpdir}
    res = run_bass_kernel_spmd(nc, maps, list(range(N_CORES)), **kw)

    out = np.empty((B, T, C), dtype=np.float32)
    for b in range(B):
        acc = res.results[4 * b]["y"].astype(np.float32)
        for hg in range(1, 4):
            acc = acc + res.results[4 * b + hg]["y"]
        out[b] = acc + b_proj[None, :]
    if _trace:
        return out, res
    return out
